# revision 10
# baseline (speedup 1.0000x reference)
"""Trainium2 Bass kernel for a transformer block (RMSNorm -> sliding-window
attention -> proj -> RMSNorm -> FFN), data-parallel over batch across 8 cores.

Per-core shapes: x [1024, 768]; sliding window 64, 12 heads of 64.

Layout strategy (per core):
  - hT / qT / kT / attn_T / h2T kept transposed [128, 6, 1024] (C on partitions)
  - scores computed transposed [keys, queries] so AV matmul (lhsT=exp_scores)
    yields the natural [q, d] layout whose softmax denominators are
    per-partition scalars (ones-column trick on v gives the sums).
  - g1/g2 and the 1/sqrt(64) scale are folded into the weights host-side.
"""

import os

import numpy as np
from contextlib import ExitStack

import concourse.bass as bass
import concourse.tile as tile
from concourse import bacc, mybir
from concourse.bass_utils import run_bass_kernel_spmd

f32 = mybir.dt.float32
AF = mybir.ActivationFunctionType
ALU = mybir.AluOpType

P = 128
T = 1024
C = 768
KC = C // P          # 6 chunks of the embedding dim
NH = 12
HD = 64
NB = T // P          # 8 t-blocks of 128
FF = 4 * C           # 3072
NF = FF // P         # 24 chunks of the FFN dim
WIN = 64
EPS = 1e-6
N_CORES = 8


def _body(tc, stop=""):
    nc = tc.nc
    x_in = nc.dram_tensor("x_in", [T, C], f32, kind="ExternalInput").ap()
    wq_d = nc.dram_tensor("wq", [C, C], f32, kind="ExternalInput").ap()
    wk_d = nc.dram_tensor("wk", [C, C], f32, kind="ExternalInput").ap()
    wv_d = nc.dram_tensor("wv", [C, C], f32, kind="ExternalInput").ap()
    wp_d = nc.dram_tensor("wp", [C, C], f32, kind="ExternalInput").ap()
    bp_d = nc.dram_tensor("bp", [C], f32, kind="ExternalInput").ap()
    w1_d = nc.dram_tensor("w1", [C, FF], f32, kind="ExternalInput").ap()
    b1_d = nc.dram_tensor("b1", [FF], f32, kind="ExternalInput").ap()
    w2_d = nc.dram_tensor("w2", [FF, C], f32, kind="ExternalInput").ap()
    b2_d = nc.dram_tensor("b2", [C], f32, kind="ExternalInput").ap()
    msk_d = nc.dram_tensor("msk", [P, 2, P], f32, kind="ExternalInput").ap()
    idn_d = nc.dram_tensor("idn", [P, P], f32, kind="ExternalInput").ap()
    y_out = nc.dram_tensor("y_out", [T, C], f32, kind="ExternalOutput").ap()

    wq_r = wq_d.rearrange("(kc p) m -> p kc m", p=P)
    wk_r = wk_d.rearrange("(kc p) m -> p kc m", p=P)
    wv_r = wv_d.rearrange("(kc p) m -> p kc m", p=P)
    wp_r = wp_d.rearrange("(kc p) m -> p kc m", p=P)
    w1_r = w1_d.rearrange("(kc p) n -> p kc n", p=P)
    w2_r = w2_d.rearrange("(kc p) n -> p kc n", p=P)
    b1_r = b1_d.rearrange("(nf p) -> p nf", p=P)

    def bcast_ap(src, n):  # partition-broadcast a [n]-vector for DMA
        return bass.AP(tensor=src.tensor, offset=src.offset, ap=[[0, P], [1, n]])

    y_dbg = y_out.rearrange("(a p) c -> p a c", p=P)

    def dump(src_tile):  # debug: dump an intermediate into y_out
        flat = src_tile.rearrange("p a b -> p (a b)").rearrange(
            "p (x y) -> p x y", x=NB)
        nc.sync.dma_start(out=y_dbg, in_=flat)

    with ExitStack() as octx:
        consts = octx.enter_context(tc.tile_pool(name="consts", bufs=1))
        ident_sb = consts.tile([P, P], f32, name="ident_sb")
        nc.sync.dma_start(out=ident_sb, in_=idn_d)
        msk_sb = consts.tile([P, 2, P], f32, name="msk_sb")
        nc.sync.dma_start(out=msk_sb, in_=msk_d)
        bp_bc = consts.tile([P, C], f32, name="bp_bc")
        nc.sync.dma_start(out=bp_bc, in_=bcast_ap(bp_d, C))
        b2_bc = consts.tile([P, C], f32, name="b2_bc")
        nc.sync.dma_start(out=b2_bc, in_=bcast_ap(b2_d, C))
        b1_sb = consts.tile([P, NF], f32, name="b1_sb")
        nc.sync.dma_start(out=b1_sb, in_=b1_r)
        eps_sb = consts.tile([P, 1], f32, name="eps_sb")
        nc.vector.memset(eps_sb, EPS)

        x2p = octx.enter_context(tc.tile_pool(name="x2p", bufs=1))
        x2 = x2p.tile([P, NB, C], f32, name="x2")

        with ExitStack() as actx:
            attn = actx.enter_context(tc.tile_pool(name="attn", bufs=1))
            hT = attn.tile([P, KC, T], f32, name="hT", tag="hT")
            qT0 = attn.tile([P, KC, T], f32, name="qT0")
            qT64 = attn.tile([P, KC, T], f32, name="qT64")
            kT = attn.tile([P, KC, T], f32, name="kT")
            v_aug = attn.tile([P, NB, NH, HD + 1], f32, name="v_aug")
            attn_T = attn.tile([P, KC, T], f32, name="attn_T", tag="hT")
            wv_sb = attn.tile([P, KC, C], f32, name="wv_sb", tag="wvp_sb")
            wp_sb = attn.tile([P, KC, C], f32, name="wp_sb", tag="wvp_sb")

            xh = actx.enter_context(tc.tile_pool(name="xh", bufs=2))
            st = actx.enter_context(tc.tile_pool(name="st", bufs=4))

            # ---- Phase A: rmsnorm(x) -> hT (transposed) ----
            with ExitStack() as pctx:
                tpp = pctx.enter_context(
                    tc.tile_pool(name="tpp", bufs=2, space="PSUM"))
                for qb in range(NB):
                    xt = xh.tile([P, C], f32, name="xt", tag="xt")
                    nc.sync.dma_start(out=xt, in_=x_in[qb * P:(qb + 1) * P, :])
                    ht = xh.tile([P, C], f32, name="ht", tag="ht")
                    ssq = st.tile([P, 1], f32, name="ssq", tag="ssq")
                    nc.scalar.activation(out=ht, in_=xt, func=AF.Square,
                                         accum_out=ssq)
                    sd = st.tile([P, 1], f32, name="sd", tag="sd")
                    nc.scalar.activation(out=sd, in_=ssq, func=AF.Sqrt,
                                         bias=eps_sb, scale=1.0 / C)
                    rstd = st.tile([P, 1], f32, name="rstd", tag="rstd")
                    nc.vector.reciprocal(out=rstd, in_=sd)
                    nc.vector.tensor_scalar_mul(out=ht, in0=xt, scalar1=rstd)
                    for c in range(KC):
                        tp = tpp.tile([P, P], f32, name="tp", tag="tp")
                        nc.tensor.transpose(tp, ht[:, c * P:(c + 1) * P],
                                            ident_sb)
                        nc.vector.tensor_copy(
                            out=hT[:, c, qb * P:(qb + 1) * P], in_=tp)

            if stop == "A":
                dump(hT)
                return
            # ---- Phase B: qT, kT, v ----
            nc.sync.dma_start(out=wv_sb, in_=wv_r)
            nc.vector.memset(v_aug[:, :, :, HD], 1.0)
            with ExitStack() as pctx:
                wpool = pctx.enter_context(tc.tile_pool(name="wpool", bufs=3))
                qkvp = pctx.enter_context(
                    tc.tile_pool(name="qkvp", bufs=2, space="PSUM"))
                vp = pctx.enter_context(
                    tc.tile_pool(name="vp", bufs=2, space="PSUM"))
                nc.vector.memset(qT0[HD:P, :, :], 0.0)
                nc.vector.memset(qT64[0:HD, :, :], 0.0)
                for w_r, dstT in ((wq_r, None), (wk_r, kT)):
                    for mt in range(KC):
                        wc = wpool.tile([P, KC, P], f32, name="wc", tag="wc")
                        nc.sync.dma_start(out=wc,
                                          in_=w_r[:, :, mt * P:(mt + 1) * P])
                        for nt in range(2):
                            ps = qkvp.tile([P, 512], f32, name="qkps",
                                           tag="qkps")
                            for kc in range(KC):
                                nc.tensor.matmul(
                                    ps, lhsT=wc[:, kc, :],
                                    rhs=hT[:, kc, nt * 512:(nt + 1) * 512],
                                    start=(kc == 0), stop=(kc == KC - 1))
                            sl = slice(nt * 512, (nt + 1) * 512)
                            if dstT is None:
                                nc.vector.tensor_copy(
                                    out=qT0[0:HD, mt, sl], in_=ps[0:HD, :])
                                nc.vector.tensor_copy(
                                    out=qT64[HD:P, mt, sl], in_=ps[HD:P, :])
                            else:
                                nc.vector.tensor_copy(
                                    out=dstT[:, mt, sl], in_=ps)
                for qb in range(NB):
                    vps = vp.tile([P, C], f32, name="vps", tag="vps")
                    for n0, n1 in ((0, 512), (512, C)):
                        for kc in range(KC):
                            nc.tensor.matmul(
                                vps[:, n0:n1],
                                lhsT=hT[:, kc, qb * P:(qb + 1) * P],
                                rhs=wv_sb[:, kc, n0:n1],
                                start=(kc == 0), stop=(kc == KC - 1))
                    nc.vector.tensor_copy(
                        out=v_aug[:, qb, :, 0:HD],
                        in_=vps.rearrange("p (h d) -> p h d", h=NH))

            if stop == "B":
                dump(qT0)
                return
            if stop == "B2":
                dump(kT)
                return
            # ---- Phase C: sliding-window attention ----
            with ExitStack() as pctx:
                sp = pctx.enter_context(
                    tc.tile_pool(name="sp", bufs=3, space="PSUM"))
                op = pctx.enter_context(
                    tc.tile_pool(name="op", bufs=3, space="PSUM"))
                tpp2 = pctx.enter_context(
                    tc.tile_pool(name="tpp2", bufs=2, space="PSUM"))
                ep = pctx.enter_context(tc.tile_pool(name="ep", bufs=7))
                apo = pctx.enter_context(tc.tile_pool(name="apo", bufs=2))
                rp = pctx.enter_context(tc.tile_pool(name="rp", bufs=4))
                for qb in range(NB):
                    chunks = ([(qb - 1, 1)] if qb > 0 else []) + [(qb, 0)]
                    etiles = {}
                    for ci, (kb, mi) in enumerate(chunks):
                        for hg in range(3):
                            sps = sp.tile([P, 4, P], f32, name="sps",
                                          tag="sps")
                            for hl in range(4):
                                h = hg * 4 + hl
                                qTp = qT64 if (h % 2) else qT0
                                pc = h // 2
                                nc.tensor.matmul(
                                    sps[:, hl, :],
                                    lhsT=kT[:, pc, kb * P:(kb + 1) * P],
                                    rhs=qTp[:, pc, qb * P:(qb + 1) * P],
                                    start=True, stop=True)
                            esb = ep.tile([P, 4, P], f32, name="esb",
                                          tag="esb")
                            nc.scalar.activation(out=esb, in_=sps,
                                                 func=AF.Exp)
                            nc.vector.tensor_tensor(
                                esb, esb,
                                msk_sb[:, mi:mi + 1, :].to_broadcast(
                                    [P, 4, P]),
                                ALU.mult)
                            etiles[(ci, hg)] = esb
                    asb = apo.tile([P, NH, HD], f32, name="asb", tag="asb")
                    for hg in range(3):
                        ops = op.tile([P, 4, P], f32, name="ops", tag="ops")
                        for hl in range(4):
                            h = hg * 4 + hl
                            for ci, (kb, mi) in enumerate(chunks):
                                nc.tensor.matmul(
                                    ops[:, hl, 0:HD + 1],
                                    lhsT=etiles[(ci, hg)][:, hl, :],
                                    rhs=v_aug[:, kb, h, :],
                                    start=(ci == 0),
                                    stop=(ci == len(chunks) - 1))
                        rc = rp.tile([P, 4], f32, name="rc", tag="rc")
                        nc.vector.reciprocal(out=rc, in_=ops[:, :, HD])
                        nc.vector.tensor_tensor(
                            asb[:, hg * 4:(hg + 1) * 4, :],
                            ops[:, :, 0:HD],
                            rc[:, :, None].to_broadcast([P, 4, HD]),
                            ALU.mult)
                    for c in range(KC):
                        tp2 = tpp2.tile([P, P], f32, name="tp2", tag="tp2")
                        nc.tensor.transpose(
                            tp2,
                            asb[:, 2 * c:2 * c + 2, :].rearrange(
                                "p a d -> p (a d)"),
                            ident_sb)
                        nc.vector.tensor_copy(
                            out=attn_T[:, c, qb * P:(qb + 1) * P], in_=tp2)

            if stop == "C":
                dump(attn_T)
                return
            # ---- Phase D: out proj + residual ----
            nc.sync.dma_start(out=wp_sb, in_=wp_r)
            with ExitStack() as pctx:
                xpp = pctx.enter_context(
                    tc.tile_pool(name="xpp", bufs=2, space="PSUM"))
                for qb in range(NB):
                    xps = xpp.tile([P, C], f32, name="xps", tag="xps")
                    for n0, n1 in ((0, 512), (512, C)):
                        for kc in range(KC):
                            nc.tensor.matmul(
                                xps[:, n0:n1],
                                lhsT=attn_T[:, kc, qb * P:(qb + 1) * P],
                                rhs=wp_sb[:, kc, n0:n1],
                                start=(kc == 0), stop=(kc == KC - 1))
                    xt2 = xh.tile([P, C], f32, name="xt2", tag="xt")
                    nc.sync.dma_start(out=xt2,
                                      in_=x_in[qb * P:(qb + 1) * P, :])
                    nc.vector.tensor_tensor(x2[:, qb, :], xps, xt2, ALU.add)
                    nc.vector.tensor_tensor(x2[:, qb, :], x2[:, qb, :],
                                            bp_bc, ALU.add)

        if stop == "D":
            dump(x2)
            return
        # ---- Phase E: rmsnorm(x2) -> h2T ----
        ffn = octx.enter_context(tc.tile_pool(name="ffn", bufs=1))
        h2T = ffn.tile([P, KC, T], f32, name="h2T")
        uT = ffn.tile([P, NF, 512], f32, name="uT")
        xh2 = octx.enter_context(tc.tile_pool(name="xh2", bufs=2))
        st2 = octx.enter_context(tc.tile_pool(name="st2", bufs=4))
        with ExitStack() as pctx:
            tpp3 = pctx.enter_context(
                tc.tile_pool(name="tpp3", bufs=2, space="PSUM"))
            for qb in range(NB):
                h2t = xh2.tile([P, C], f32, name="h2t", tag="h2t")
                ssq2 = st2.tile([P, 1], f32, name="ssq2", tag="ssq2")
                nc.scalar.activation(out=h2t, in_=x2[:, qb, :],
                                     func=AF.Square, accum_out=ssq2)
                sd2 = st2.tile([P, 1], f32, name="sd2", tag="sd2")
                nc.scalar.activation(out=sd2, in_=ssq2, func=AF.Sqrt,
                                     bias=eps_sb, scale=1.0 / C)
                rstd2 = st2.tile([P, 1], f32, name="rstd2", tag="rstd2")
                nc.vector.reciprocal(out=rstd2, in_=sd2)
                nc.vector.tensor_scalar_mul(out=h2t, in0=x2[:, qb, :],
                                            scalar1=rstd2)
                for c in range(KC):
                    tp3 = tpp3.tile([P, P], f32, name="tp3", tag="tp3")
                    nc.tensor.transpose(tp3, h2t[:, c * P:(c + 1) * P],
                                        ident_sb)
                    nc.vector.tensor_copy(
                        out=h2T[:, c, qb * P:(qb + 1) * P], in_=tp3)

        if stop == "E":
            dump(h2T)
            return
        # ---- Phase F: FFN + residual, in two t-halves ----
        w1p = octx.enter_context(tc.tile_pool(name="w1p", bufs=3))
        w2p = octx.enter_context(tc.tile_pool(name="w2p", bufs=4))
        outp = octx.enter_context(tc.tile_pool(name="outp", bufs=3))
        for half in range(2):
            t0 = half * 512
            with ExitStack() as pctx:
                up = pctx.enter_context(
                    tc.tile_pool(name=f"up{half}", bufs=2, space="PSUM"))
                for nf in range(NF):
                    w1c = w1p.tile([P, KC, P], f32, name="w1c", tag="w1c")
                    nc.sync.dma_start(out=w1c,
                                      in_=w1_r[:, :, nf * P:(nf + 1) * P])
                    ups = up.tile([P, 512], f32, name="ups", tag="ups")
                    for kc in range(KC):
                        nc.tensor.matmul(ups, lhsT=w1c[:, kc, :],
                                         rhs=h2T[:, kc, t0:t0 + 512],
                                         start=(kc == 0), stop=(kc == KC - 1))
                    nc.scalar.activation(out=uT[:, nf, :], in_=ups,
                                         func=AF.Gelu_apprx_tanh,
                                         bias=b1_sb[:, nf:nf + 1])
            with ExitStack() as pctx:
                yp = pctx.enter_context(
                    tc.tile_pool(name=f"yp{half}", bufs=1, space="PSUM"))
                ypss = [yp.tile([P, C], f32, name=f"yps{ql}", tag=f"yps{ql}")
                        for ql in range(4)]
                for kc in range(NF):
                    w2c = w2p.tile([P, C], f32, name="w2c", tag="w2c")
                    nc.sync.dma_start(out=w2c, in_=w2_r[:, kc, :])
                    for ql in range(4):
                        for n0, n1 in ((0, 512), (512, C)):
                            nc.tensor.matmul(
                                ypss[ql][:, n0:n1],
                                lhsT=uT[:, kc, ql * P:(ql + 1) * P],
                                rhs=w2c[:, n0:n1],
                                start=(kc == 0), stop=(kc == NF - 1))
                for ql in range(4):
                    qb = half * 4 + ql
                    ot = outp.tile([P, C], f32, name="ot", tag="ot")
                    nc.vector.tensor_tensor(ot, ypss[ql], x2[:, qb, :],
                                            ALU.add)
                    nc.vector.tensor_tensor(ot, ot, b2_bc, ALU.add)
                    nc.sync.dma_start(out=y_out[qb * P:(qb + 1) * P, :],
                                      in_=ot)


_NC_CACHE = None


def build_program():
    global _NC_CACHE
    if _NC_CACHE is not None:
        return _NC_CACHE
    nc = bacc.Bacc("TRN2", target_bir_lowering=False, debug=False,
                   enable_asserts=False)
    with tile.TileContext(nc) as tc:
        _body(tc, stop=os.environ.get("K_STOP", ""))
    nc.compile()
    _NC_CACHE = nc
    return nc


def make_in_maps(x, w_q, w_k, w_v, w_proj, b_proj, w_ff1, b_ff1, w_ff2,
                 b_ff2, g1, g2):
    x = np.ascontiguousarray(np.asarray(x, dtype=np.float32))
    g1 = np.asarray(g1, dtype=np.float32)
    g2 = np.asarray(g2, dtype=np.float32)
    wq = np.ascontiguousarray(
        (g1[:, None] * np.asarray(w_q, np.float32).reshape(C, C))
        * (1.0 / np.sqrt(HD)))
    wk = np.ascontiguousarray(
        g1[:, None] * np.asarray(w_k, np.float32).reshape(C, C))
    wv = np.ascontiguousarray(
        g1[:, None] * np.asarray(w_v, np.float32).reshape(C, C))
    wp = np.ascontiguousarray(np.asarray(w_proj, np.float32))
    bp = np.ascontiguousarray(np.asarray(b_proj, np.float32))
    w1 = np.ascontiguousarray(g2[:, None] * np.asarray(w_ff1, np.float32))
    b1 = np.ascontiguousarray(np.asarray(b_ff1, np.float32))
    w2 = np.ascontiguousarray(np.asarray(w_ff2, np.float32))
    b2 = np.ascontiguousarray(np.asarray(b_ff2, np.float32))

    f = np.arange(P)
    diff = f[None, :] - f[:, None]          # [key_p, query_f] = f - p
    mask_diag = ((diff >= 0) & (diff < WIN)).astype(np.float32)
    mask_prev = (-diff >= 65).astype(np.float32)
    msk = np.ascontiguousarray(
        np.stack([mask_diag, mask_prev], axis=1))  # [P, 2, P]
    idn = np.eye(P, dtype=np.float32)

    shared = dict(wq=wq, wk=wk, wv=wv, wp=wp, bp=bp, w1=w1, b1=b1, w2=w2,
                  b2=b2, msk=msk, idn=idn)
    return [dict(x_in=np.ascontiguousarray(x[b]), **shared)
            for b in range(N_CORES)]


def run(in_maps, trace=False, **kwargs):
    nc = build_program()
    return run_bass_kernel_spmd(nc, in_maps, core_ids=list(range(N_CORES)),
                                trace=trace, **kwargs)


def kernel(x, w_q, w_k, w_v, w_proj, b_proj, w_ff1, b_ff1, w_ff2, b_ff2,
           g1, g2):
    in_maps = make_in_maps(x, w_q, w_k, w_v, w_proj, b_proj, w_ff1, b_ff1,
                           w_ff2, b_ff2, g1, g2)
    res = run(in_maps, trace=False)
    out = np.stack([res.results[b]["y_out"] for b in range(N_CORES)], axis=0)
    return out, np.float32(0.0)


# revision 13
# speedup vs baseline: 2.5636x; 2.5636x over previous
"""Trainium2 Bass kernel for a transformer block (RMSNorm -> sliding-window
attention -> proj -> RMSNorm -> FFN), data-parallel over batch across 8 cores.

Per-core shapes: x [1024, 768]; sliding window 64, 12 heads of 64.

Layout strategy (per core):
  - hT / qT / kT / attn_T / h2T kept transposed [128, 6, 1024] (C on partitions)
  - scores computed transposed [keys, queries] so AV matmul (lhsT=exp_scores)
    yields the natural [q, d] layout whose softmax denominators are
    per-partition scalars (ones-column trick on v gives the sums).
  - g1/g2 and the 1/sqrt(64) scale are folded into the weights host-side.
"""

import os

import numpy as np
from contextlib import ExitStack

import concourse.bass as bass
import concourse.tile as tile
from concourse import bacc, mybir
from concourse.bass_utils import run_bass_kernel_spmd

f32 = mybir.dt.float32
f32r = mybir.dt.float32r
AF = mybir.ActivationFunctionType
ALU = mybir.AluOpType

P = 128
T = 1024
C = 768
KC = C // P          # 6 chunks of the embedding dim
NH = 12
HD = 64
NB = T // P          # 8 t-blocks of 128
FF = 4 * C           # 3072
NF = FF // P         # 24 chunks of the FFN dim
WIN = 64
EPS = 1e-6
N_CORES = 8


def _body(tc, stop=""):
    nc = tc.nc
    x_in = nc.dram_tensor("x_in", [T, C], f32, kind="ExternalInput").ap()
    wq_d = nc.dram_tensor("wq", [C, C], f32r, kind="ExternalInput").ap()
    wk_d = nc.dram_tensor("wk", [C, C], f32r, kind="ExternalInput").ap()
    wv_d = nc.dram_tensor("wv", [C, C], f32r, kind="ExternalInput").ap()
    wp_d = nc.dram_tensor("wp", [C, C], f32r, kind="ExternalInput").ap()
    bp_d = nc.dram_tensor("bp", [C], f32, kind="ExternalInput").ap()
    w1_d = nc.dram_tensor("w1", [C, FF], f32r, kind="ExternalInput").ap()
    b1_d = nc.dram_tensor("b1", [FF], f32, kind="ExternalInput").ap()
    w2_d = nc.dram_tensor("w2", [FF, C], f32r, kind="ExternalInput").ap()
    b2_d = nc.dram_tensor("b2", [C], f32, kind="ExternalInput").ap()
    msk_d = nc.dram_tensor("msk", [P, 2, P], f32, kind="ExternalInput").ap()
    z0_d = nc.dram_tensor("z0", [KC * T], f32r, kind="ExternalInput").ap()
    idn_d = nc.dram_tensor("idn", [P, P], f32, kind="ExternalInput").ap()
    y_out = nc.dram_tensor("y_out", [T, C], f32, kind="ExternalOutput").ap()

    wq_r = wq_d.rearrange("(kc p) m -> p kc m", p=P)
    wk_r = wk_d.rearrange("(kc p) m -> p kc m", p=P)
    wv_r = wv_d.rearrange("(kc p) m -> p kc m", p=P)
    wp_r = wp_d.rearrange("(kc p) m -> p kc m", p=P)
    w1_r = w1_d.rearrange("(kc p) n -> p kc n", p=P)
    w2_r = w2_d.rearrange("(kc p) n -> p kc n", p=P)
    b1_r = b1_d.rearrange("(nf p) -> p nf", p=P)

    def bcast_ap(src, n):  # partition-broadcast a [n]-vector for DMA
        return bass.AP(tensor=src.tensor, offset=src.offset, ap=[[0, P], [1, n]])

    y_dbg = y_out.rearrange("(a p) c -> p a c", p=P)

    def dump(src_tile):  # debug: dump an intermediate into y_out
        flat = src_tile.rearrange("p a b -> p (a b)").rearrange(
            "p (x y) -> p x y", x=NB)
        nc.sync.dma_start(out=y_dbg, in_=flat)

    with ExitStack() as octx:
        consts = octx.enter_context(tc.tile_pool(name="consts", bufs=1))
        ident_sb = consts.tile([P, P], f32, name="ident_sb")
        nc.sync.dma_start(out=ident_sb, in_=idn_d)
        msk_sb = consts.tile([P, 2, P], f32, name="msk_sb")
        nc.sync.dma_start(out=msk_sb, in_=msk_d)
        bp_bc = consts.tile([P, C], f32, name="bp_bc")
        nc.sync.dma_start(out=bp_bc, in_=bcast_ap(bp_d, C))
        b2_bc = consts.tile([P, C], f32, name="b2_bc")
        nc.sync.dma_start(out=b2_bc, in_=bcast_ap(b2_d, C))
        b1_sb = consts.tile([P, NF], f32, name="b1_sb")
        nc.sync.dma_start(out=b1_sb, in_=b1_r)
        eps_sb = consts.tile([P, 1], f32, name="eps_sb")
        nc.vector.memset(eps_sb, EPS)

        x2p = octx.enter_context(tc.tile_pool(name="x2p", bufs=1))
        x2 = x2p.tile([P, NB, C], f32, name="x2")

        with ExitStack() as actx:
            attn = actx.enter_context(tc.tile_pool(name="attn", bufs=1))
            hT = attn.tile([P, KC, T], f32r, name="hT", tag="hT")
            qT0 = attn.tile([P, KC, T], f32r, name="qT0")
            qT64 = attn.tile([P, KC, T], f32r, name="qT64")
            kT = attn.tile([P, KC, T], f32r, name="kT")
            v_aug = attn.tile([P, NB, NH, HD + 1], f32, name="v_aug")
            attn_T = attn.tile([P, KC, T], f32r, name="attn_T", tag="hT")
            wv_sb = attn.tile([P, KC, C], f32r, name="wv_sb", tag="wvp_sb")
            wp_sb = attn.tile([P, KC, C], f32r, name="wp_sb", tag="wvp_sb")

            st = actx.enter_context(tc.tile_pool(name="st", bufs=4))

            # ---- Phase A: rmsnorm(x) -> hT (transposed) ----
            with ExitStack() as pctx:
                xh = pctx.enter_context(tc.tile_pool(name="xh", bufs=2))
                tpp = pctx.enter_context(
                    tc.tile_pool(name="tpp", bufs=2, space="PSUM"))
                for qb in range(NB):
                    xt = xh.tile([P, C], f32, name="xt", tag="xt")
                    nc.sync.dma_start(out=xt, in_=x_in[qb * P:(qb + 1) * P, :])
                    ht = xh.tile([P, C], f32, name="ht", tag="ht")
                    ssq = st.tile([P, 1], f32, name="ssq", tag="ssq")
                    nc.scalar.activation(out=ht, in_=xt, func=AF.Square,
                                         accum_out=ssq)
                    sd = st.tile([P, 1], f32, name="sd", tag="sd")
                    nc.scalar.activation(out=sd, in_=ssq, func=AF.Sqrt,
                                         bias=eps_sb, scale=1.0 / C)
                    rstd = st.tile([P, 1], f32, name="rstd", tag="rstd")
                    nc.vector.reciprocal(out=rstd, in_=sd)
                    nc.vector.tensor_scalar_mul(out=ht, in0=xt, scalar1=rstd)
                    for c in range(KC):
                        tp = tpp.tile([P, P], f32, name="tp", tag="tp")
                        nc.tensor.transpose(tp, ht[:, c * P:(c + 1) * P],
                                            ident_sb)
                        nc.vector.tensor_copy(
                            out=hT[:, c, qb * P:(qb + 1) * P], in_=tp)

            if stop == "A":
                dump(hT)
                return
            # ---- Phase B: qT, kT, v ----
            nc.sync.dma_start(out=wv_sb, in_=wv_r)
            nc.vector.memset(v_aug[:, :, :, HD], 1.0)
            with ExitStack() as pctx:
                wpool = pctx.enter_context(tc.tile_pool(name="wpool", bufs=3))
                qkvp = pctx.enter_context(
                    tc.tile_pool(name="qkvp", bufs=2, space="PSUM"))
                vp = pctx.enter_context(
                    tc.tile_pool(name="vp", bufs=2, space="PSUM"))
                zsrc = bass.AP(tensor=z0_d.tensor, offset=0,
                               ap=[[0, HD], [T, KC], [1, T]])
                nc.sync.dma_start(out=qT0[HD:P, :, :], in_=zsrc)
                nc.sync.dma_start(out=qT64[0:HD, :, :], in_=zsrc)
                for w_r, dstT in ((wq_r, None), (wk_r, kT)):
                    for mt in range(KC):
                        wc = wpool.tile([P, KC, P], f32r, name="wc", tag="wc")
                        nc.sync.dma_start(out=wc,
                                          in_=w_r[:, :, mt * P:(mt + 1) * P])
                        for nt in range(2):
                            ps = qkvp.tile([P, 512], f32, name="qkps",
                                           tag="qkps")
                            for kc in range(KC):
                                nc.tensor.matmul(
                                    ps, lhsT=wc[:, kc, :],
                                    rhs=hT[:, kc, nt * 512:(nt + 1) * 512],
                                    start=(kc == 0), stop=(kc == KC - 1))
                            sl = slice(nt * 512, (nt + 1) * 512)
                            if dstT is None:
                                nc.vector.tensor_copy(
                                    out=qT0[0:HD, mt, sl], in_=ps[0:HD, :])
                                nc.vector.tensor_copy(
                                    out=qT64[HD:P, mt, sl], in_=ps[HD:P, :])
                            else:
                                nc.vector.tensor_copy(
                                    out=dstT[:, mt, sl], in_=ps)
                for qb in range(NB):
                    vps = vp.tile([P, C], f32, name="vps", tag="vps")
                    for n0, n1 in ((0, 512), (512, C)):
                        for kc in range(KC):
                            nc.tensor.matmul(
                                vps[:, n0:n1],
                                lhsT=hT[:, kc, qb * P:(qb + 1) * P],
                                rhs=wv_sb[:, kc, n0:n1],
                                start=(kc == 0), stop=(kc == KC - 1))
                    nc.vector.tensor_copy(
                        out=v_aug[:, qb, :, 0:HD],
                        in_=vps.rearrange("p (h d) -> p h d", h=NH))

            if stop == "B":
                dump(qT0)
                return
            if stop == "B2":
                dump(kT)
                return
            # ---- Phase C: sliding-window attention ----
            # Window c: keys = chunk c, queries = [c*128, c*128+256).
            # First 128 queries are the diagonal block (mask 0), second 128
            # are the next q-block for which chunk c is the previous block
            # (mask 1).
            with ExitStack() as pctx:
                sp = pctx.enter_context(
                    tc.tile_pool(name="sp", bufs=2, space="PSUM"))
                op = pctx.enter_context(
                    tc.tile_pool(name="op", bufs=2, space="PSUM"))
                tpp2 = pctx.enter_context(
                    tc.tile_pool(name="tpp2", bufs=2, space="PSUM"))
                ep = pctx.enter_context(tc.tile_pool(name="ep", bufs=7))
                apo = pctx.enter_context(tc.tile_pool(name="apo", bufs=2))
                rp = pctx.enter_context(tc.tile_pool(name="rp", bufs=4))
                etiles = {}
                for c in range(NB):
                    wlen = 256 if c < NB - 1 else 128
                    for hg in range(3):
                        sps = sp.tile([P, 4, 256], f32, name="sps",
                                      tag="sps")
                        for hl in range(4):
                            h = hg * 4 + hl
                            qTp = qT64 if (h % 2) else qT0
                            pc = h // 2
                            nc.tensor.matmul(
                                sps[:, hl, 0:wlen],
                                lhsT=kT[:, pc, c * P:(c + 1) * P],
                                rhs=qTp[:, pc, c * P:c * P + wlen],
                                start=True, stop=True)
                        esb = ep.tile([P, 4, 256], f32, name="esb",
                                      tag="esb")
                        nc.scalar.activation(out=esb[:, :, 0:wlen],
                                             in_=sps[:, :, 0:wlen],
                                             func=AF.Exp)
                        if wlen == 256:
                            ev = esb.rearrange("p a (w q) -> p a w q", w=2)
                            nc.vector.tensor_tensor(
                                ev, ev,
                                msk_sb[:, None, :, :].to_broadcast(
                                    [P, 4, 2, P]),
                                ALU.mult)
                        else:
                            nc.vector.tensor_tensor(
                                esb[:, :, 0:P], esb[:, :, 0:P],
                                msk_sb[:, 0:1, :].to_broadcast([P, 4, P]),
                                ALU.mult)
                        etiles[(c, hg)] = esb
                    qb = c
                    asb = apo.tile([P, NH, HD], f32, name="asb", tag="asb")
                    for hg in range(3):
                        ops = op.tile([P, 4, P], f32, name="ops", tag="ops")
                        for hl in range(4):
                            h = hg * 4 + hl
                            srcs = []
                            if qb > 0:
                                srcs.append((etiles[(qb - 1, hg)],
                                             slice(P, 2 * P), qb - 1))
                            srcs.append((etiles[(qb, hg)], slice(0, P), qb))
                            for ci, (et, sl, kb) in enumerate(srcs):
                                nc.tensor.matmul(
                                    ops[:, hl, 0:HD + 1],
                                    lhsT=et[:, hl, sl],
                                    rhs=v_aug[:, kb, h, :],
                                    start=(ci == 0),
                                    stop=(ci == len(srcs) - 1))
                        rc = rp.tile([P, 4], f32, name="rc", tag="rc")
                        nc.vector.reciprocal(out=rc, in_=ops[:, :, HD])
                        nc.vector.tensor_tensor(
                            asb[:, hg * 4:(hg + 1) * 4, :],
                            ops[:, :, 0:HD],
                            rc[:, :, None].to_broadcast([P, 4, HD]),
                            ALU.mult)
                    for cc in range(KC):
                        tp2 = tpp2.tile([P, P], f32, name="tp2", tag="tp2")
                        nc.tensor.transpose(
                            tp2,
                            asb[:, 2 * cc:2 * cc + 2, :].rearrange(
                                "p a d -> p (a d)"),
                            ident_sb)
                        nc.vector.tensor_copy(
                            out=attn_T[:, cc, qb * P:(qb + 1) * P],
                            in_=tp2)

            if stop == "C":
                dump(attn_T)
                return
            # ---- Phase D: out proj + residual ----
            nc.sync.dma_start(out=wp_sb, in_=wp_r)
            with ExitStack() as pctx:
                xh = pctx.enter_context(tc.tile_pool(name="xh", bufs=2))
                xpp = pctx.enter_context(
                    tc.tile_pool(name="xpp", bufs=2, space="PSUM"))
                for qb in range(NB):
                    xps = xpp.tile([P, C], f32, name="xps", tag="xps")
                    for n0, n1 in ((0, 512), (512, C)):
                        for kc in range(KC):
                            nc.tensor.matmul(
                                xps[:, n0:n1],
                                lhsT=attn_T[:, kc, qb * P:(qb + 1) * P],
                                rhs=wp_sb[:, kc, n0:n1],
                                start=(kc == 0), stop=(kc == KC - 1))
                    xt2 = xh.tile([P, C], f32, name="xt2", tag="xt")
                    nc.sync.dma_start(out=xt2,
                                      in_=x_in[qb * P:(qb + 1) * P, :])
                    nc.vector.tensor_tensor(x2[:, qb, :], xps, xt2, ALU.add)
                    nc.vector.tensor_tensor(x2[:, qb, :], x2[:, qb, :],
                                            bp_bc, ALU.add)

        if stop == "D":
            dump(x2)
            return
        # ---- Phase E: rmsnorm(x2) -> h2T ----
        ffn = octx.enter_context(tc.tile_pool(name="ffn", bufs=1))
        h2T = ffn.tile([P, KC, T], f32r, name="h2T")
        uT = ffn.tile([P, NF, 512], f32r, name="uT")
        xh2 = octx.enter_context(tc.tile_pool(name="xh2", bufs=2))
        st2 = octx.enter_context(tc.tile_pool(name="st2", bufs=4))
        with ExitStack() as pctx:
            tpp3 = pctx.enter_context(
                tc.tile_pool(name="tpp3", bufs=2, space="PSUM"))
            for qb in range(NB):
                h2t = xh2.tile([P, C], f32, name="h2t", tag="h2t")
                ssq2 = st2.tile([P, 1], f32, name="ssq2", tag="ssq2")
                nc.scalar.activation(out=h2t, in_=x2[:, qb, :],
                                     func=AF.Square, accum_out=ssq2)
                sd2 = st2.tile([P, 1], f32, name="sd2", tag="sd2")
                nc.scalar.activation(out=sd2, in_=ssq2, func=AF.Sqrt,
                                     bias=eps_sb, scale=1.0 / C)
                rstd2 = st2.tile([P, 1], f32, name="rstd2", tag="rstd2")
                nc.vector.reciprocal(out=rstd2, in_=sd2)
                nc.vector.tensor_scalar_mul(out=h2t, in0=x2[:, qb, :],
                                            scalar1=rstd2)
                for c in range(KC):
                    tp3 = tpp3.tile([P, P], f32, name="tp3", tag="tp3")
                    nc.tensor.transpose(tp3, h2t[:, c * P:(c + 1) * P],
                                        ident_sb)
                    nc.vector.tensor_copy(
                        out=h2T[:, c, qb * P:(qb + 1) * P], in_=tp3)

        if stop == "E":
            dump(h2T)
            return
        # ---- Phase F: FFN + residual, in two t-halves ----
        w1p = octx.enter_context(tc.tile_pool(name="w1p", bufs=3))
        w2p = octx.enter_context(tc.tile_pool(name="w2p", bufs=4))
        outp = octx.enter_context(tc.tile_pool(name="outp", bufs=3))
        for half in range(2):
            t0 = half * 512
            with ExitStack() as pctx:
                up = pctx.enter_context(
                    tc.tile_pool(name=f"up{half}", bufs=2, space="PSUM"))
                for nf in range(NF):
                    w1c = w1p.tile([P, KC, P], f32r, name="w1c", tag="w1c")
                    nc.sync.dma_start(out=w1c,
                                      in_=w1_r[:, :, nf * P:(nf + 1) * P])
                    ups = up.tile([P, 512], f32, name="ups", tag="ups")
                    for kc in range(KC):
                        nc.tensor.matmul(ups, lhsT=w1c[:, kc, :],
                                         rhs=h2T[:, kc, t0:t0 + 512],
                                         start=(kc == 0), stop=(kc == KC - 1))
                    nc.scalar.activation(out=uT[:, nf, :], in_=ups,
                                         func=AF.Gelu_apprx_tanh,
                                         bias=b1_sb[:, nf:nf + 1])
            with ExitStack() as pctx:
                yp = pctx.enter_context(
                    tc.tile_pool(name=f"yp{half}", bufs=1, space="PSUM"))
                ypss = [yp.tile([P, C], f32, name=f"yps{ql}", tag=f"yps{ql}")
                        for ql in range(4)]
                for kc in range(NF):
                    w2c = w2p.tile([P, C], f32r, name="w2c", tag="w2c")
                    nc.sync.dma_start(out=w2c, in_=w2_r[:, kc, :])
                    for ql in range(4):
                        for n0, n1 in ((0, 512), (512, C)):
                            nc.tensor.matmul(
                                ypss[ql][:, n0:n1],
                                lhsT=uT[:, kc, ql * P:(ql + 1) * P],
                                rhs=w2c[:, n0:n1],
                                start=(kc == 0), stop=(kc == NF - 1))
                for ql in range(4):
                    qb = half * 4 + ql
                    ot = outp.tile([P, C], f32, name="ot", tag="ot")
                    nc.vector.tensor_tensor(ot, ypss[ql], x2[:, qb, :],
                                            ALU.add)
                    nc.vector.tensor_tensor(ot, ot, b2_bc, ALU.add)
                    nc.sync.dma_start(out=y_out[qb * P:(qb + 1) * P, :],
                                      in_=ot)


_NC_CACHE = None


def build_program():
    global _NC_CACHE
    if _NC_CACHE is not None:
        return _NC_CACHE
    nc = bacc.Bacc("TRN2", target_bir_lowering=False, debug=False,
                   enable_asserts=False)
    with tile.TileContext(nc) as tc:
        _body(tc, stop=os.environ.get("K_STOP", ""))
    nc.compile()
    _NC_CACHE = nc
    return nc


def _round_f32r(x):
    u = np.ascontiguousarray(np.asarray(x, np.float32)).view(np.uint32)
    r = (u + 0x7FF + ((u >> 12) & 1)) & np.uint32(0xFFFFF000)
    return r.view(np.float32)


def make_in_maps(x, w_q, w_k, w_v, w_proj, b_proj, w_ff1, b_ff1, w_ff2,
                 b_ff2, g1, g2):
    x = np.ascontiguousarray(np.asarray(x, dtype=np.float32))
    g1 = np.asarray(g1, dtype=np.float32)
    g2 = np.asarray(g2, dtype=np.float32)
    wq = np.ascontiguousarray(
        (g1[:, None] * np.asarray(w_q, np.float32).reshape(C, C))
        * (1.0 / np.sqrt(HD)))
    wk = np.ascontiguousarray(
        g1[:, None] * np.asarray(w_k, np.float32).reshape(C, C))
    wv = np.ascontiguousarray(
        g1[:, None] * np.asarray(w_v, np.float32).reshape(C, C))
    wp = np.ascontiguousarray(np.asarray(w_proj, np.float32))
    bp = np.ascontiguousarray(np.asarray(b_proj, np.float32))
    w1 = np.ascontiguousarray(g2[:, None] * np.asarray(w_ff1, np.float32))
    b1 = np.ascontiguousarray(np.asarray(b_ff1, np.float32))
    w2 = np.ascontiguousarray(np.asarray(w_ff2, np.float32))
    b2 = np.ascontiguousarray(np.asarray(b_ff2, np.float32))

    f = np.arange(P)
    diff = f[None, :] - f[:, None]          # [key_p, query_f] = f - p
    mask_diag = ((diff >= 0) & (diff < WIN)).astype(np.float32)
    mask_prev = (-diff >= 65).astype(np.float32)
    msk = np.ascontiguousarray(
        np.stack([mask_diag, mask_prev], axis=1))  # [P, 2, P]
    idn = np.eye(P, dtype=np.float32)

    shared = dict(wq=_round_f32r(wq), wk=_round_f32r(wk), wv=_round_f32r(wv),
                  wp=_round_f32r(wp), bp=bp, w1=_round_f32r(w1), b1=b1,
                  w2=_round_f32r(w2), b2=b2, msk=msk, idn=idn,
                  z0=np.zeros(KC * T, np.float32))
    return [dict(x_in=np.ascontiguousarray(x[b]), **shared)
            for b in range(N_CORES)]


def run(in_maps, trace=False, **kwargs):
    nc = build_program()
    return run_bass_kernel_spmd(nc, in_maps, core_ids=list(range(N_CORES)),
                                trace=trace, **kwargs)


def kernel(x, w_q, w_k, w_v, w_proj, b_proj, w_ff1, b_ff1, w_ff2, b_ff2,
           g1, g2):
    in_maps = make_in_maps(x, w_q, w_k, w_v, w_proj, b_proj, w_ff1, b_ff1,
                           w_ff2, b_ff2, g1, g2)
    res = run(in_maps, trace=False)
    out = np.stack([res.results[b]["y_out"] for b in range(N_CORES)], axis=0)
    return out, np.float32(0.0)
